# revision 22
# baseline (speedup 1.0000x reference)
"""Causal multi-head attention + RoPE — Trainium2 Bass kernel, 8-core SPMD.

Sharding: batch (2) x head-groups (4 heads each) -> 8 cores.
Wq/Wk/Wv are column-sharded per head group, Wo row-sharded; each core
computes a partial out-projection [S, D] and the host sums the 4
partials per batch (the "all-reduce after out_proj").

Per-core device pipeline (matmul inputs bf16, fp32 PSUM accumulate):
  1. Q^T/K^T = Wq'^T.T @ x^T with host-permuted weight rows so results
     land as [even-features(128) ; odd-features(128)] x [S] tiles ->
     full-width RoPE on VectorE with no transposes; then 0/1-matrix
     permutation matmuls regroup rows head-contiguously for K=64
     score matmuls.  V = x @ Wv^T in natural layout with a ones
     column appended per head ([V_h | 1]).
  2. Per head pair: transposed scores S^T[k,q] = K_h^T.T @ Q_h^T with
     the two heads on PE row-groups 0/64 (tile_position) so they
     overlap on the array; k-blocks two at a time into a 2-bank PSUM
     tile, one ScalarE Exp per tile (scores are bounded, max-pass
     free); causal via per-block q-trimming + a triangular mask on
     diagonal blocks; O^T[f,q] plus a denominator row (from the ones
     column) accumulate over k in PSUM.
  3. Softmax division: VectorE reciprocal -> K=1 float32r matmul
     broadcast across partitions -> VectorE multiply, emitted one
     head-pair late so the chain never stalls the in-order PE queue.
  4. partial = O_norm @ Wo_shard^T per 512-row q-slab, emitted as
     soon as that slab's divisions are in.
Projection/RoPE/regroup work is interleaved between attention steps
as PE filler (deferred-copy scheme keeps ScalarE's queue head free),
so the ScalarE-bound softmax and PE-bound projections overlap.
"""

import os
import sys
from contextlib import ExitStack

import numpy as np

for _p in ("/opt/trn_rl_repo", "/root/.axon_site/_ro/trn_rl_repo"):
    if os.path.isdir(_p) and _p not in sys.path:
        sys.path.insert(0, _p)

import ml_dtypes  # noqa: E402
import concourse.bass as bass  # noqa: E402
import concourse.tile as tile  # noqa: E402
from concourse import bacc, mybir  # noqa: E402
from concourse.bass_utils import run_bass_kernel_spmd  # noqa: E402

BF16 = mybir.dt.bfloat16
F32 = mybir.dt.float32
F32R = mybir.dt.float32r
AF = mybir.ActivationFunctionType

B, S, D = 2, 2048, 1024
H, DK = 16, 64
HPC = 4                # heads per core
HF = HPC * DK          # 256 projected features per core
N_CORES = 8
THETA = 10000.0
SCALE = 1.0 / (DK ** 0.5)

KD = D // 128          # 8 contraction chunks for projections
NS = S // 512          # 4 q-ranges of 512
SB = S // 128          # 16 s-blocks of 128


# ---------------------------------------------------------------------------
# Device program (identical on all 8 cores; only the input shards differ)
# ---------------------------------------------------------------------------
def _build_program():
    # bacc.Bacc (not bass.Bass): its compile() pipeline runs
    # generate_event_semaphores, which splits multi-sem waits to satisfy
    # the 1-wait-per-instruction TRN2 constraint in walrus codegen.
    nc = bacc.Bacc("TRN2", target_bir_lowering=False, debug=False,
                   num_devices=N_CORES)

    # consolidated inputs: one DMA each (HWDGE costs ~625ns per DMA)
    xT = nc.dram_tensor("xT", [D, S], BF16, kind="ExternalInput")
    wqA = nc.dram_tensor("wqA", [128, KD * HF], BF16, kind="ExternalInput")
    wkA = nc.dram_tensor("wkA", [128, KD * HF], BF16, kind="ExternalInput")
    wvA = nc.dram_tensor("wvA", [128, KD * HF], BF16, kind="ExternalInput")
    woA = nc.dram_tensor("woA", [128, 2 * D], BF16, kind="ExternalInput")
    csA = nc.dram_tensor("csA", [128, 2 * S], BF16, kind="ExternalInput")
    # [tri(128) | Pe0 Po0 Pe1 Pe1(4x128)] packed as one bf16 tensor
    cstA = nc.dram_tensor("cstA", [128, 5 * 128], BF16, kind="ExternalInput")
    onesT = nc.dram_tensor("onesT", [1, 64], F32R, kind="ExternalInput")
    out = nc.dram_tensor("out", [S, D], BF16, kind="ExternalOutput")

    with tile.TileContext(nc) as tc, ExitStack() as ctx:
        cons = ctx.enter_context(tc.tile_pool(name="cons", bufs=1))

        # ---- persistent SBUF tensors -----------------------------------
        xt = [cons.tile([128, S], BF16, tag=f"xt{k}", name=f"xt{k}")
              for k in range(KD)]
        wqa = cons.tile([128, KD * HF], BF16, tag="wqa", name="wqa")
        wka = cons.tile([128, KD * HF], BF16, tag="wka", name="wka")
        wva = cons.tile([128, KD * HF], BF16, tag="wva", name="wva")
        woa = cons.tile([128, 2 * D], BF16, tag="woa", name="woa")
        csa = cons.tile([128, 2 * S], BF16, tag="csa", name="csa")
        cos_t, sin_t = csa[:, 0:S], csa[:, S:2 * S]
        csta = cons.tile([128, 5 * 128], BF16, tag="csta", name="csta")
        tri_t = csta[:, 0:128]
        perm = [csta[:, 128 * (i + 1):128 * (i + 2)] for i in range(4)]
        ones_t = cons.tile([1, 64], F32R, tag="ones", name="ones")
        # bf16 pre-RoPE staging and bf16 post-RoPE Q^T/K^T halves
        qf = [cons.tile([128, S], BF16, tag=f"qf{m}", name=f"qf{m}")
              for m in range(2)]
        kf = [cons.tile([128, S], BF16, tag=f"kf{m}", name=f"kf{m}")
              for m in range(2)]
        qte = cons.tile([128, S], BF16, tag="qte", name="qte")
        qto = cons.tile([128, S], BF16, tag="qto", name="qto")
        kte = cons.tile([128, S], BF16, tag="kte", name="kte")
        kto = cons.tile([128, S], BF16, tag="kto", name="kto")
        # head-contiguous Q^T/K^T (64 features per head) for K=64 scores
        qc = [cons.tile([128, S], BF16, tag=f"qc{i}", name=f"qc{i}")
              for i in range(2)]
        kc = [cons.tile([128, S], BF16, tag=f"kc{i}", name=f"kc{i}")
              for i in range(2)]
        # V in natural layout, 65 columns per head (ones appended)
        v_sb = [cons.tile([128, HPC * (DK + 1)], BF16, tag=f"v{s}",
                          name=f"v{s}")
                for s in range(SB)]
        # normalized O^T (features x S), two 128-feature tiles
        otb = [cons.tile([128, S], BF16, tag=f"otb{i}", name=f"otb{i}")
               for i in range(2)]

        # input DMAs, ordered by first use (one serialized DMA device in
        # the cost model, so order = arrival schedule)
        nc.sync.dma_start(wqa[:], wqA[:, :])
        for k in range(KD):
            nc.sync.dma_start(xt[k][:], xT[128 * k:128 * (k + 1), :])
        nc.sync.dma_start(wka[:], wkA[:, :])
        # rope(0) needs only the first 512 cols of cos/sin: land those first
        nc.sync.dma_start(csa[:, 0:512], csA[:, 0:512])
        nc.sync.dma_start(csa[:, S:S + 512], csA[:, S:S + 512])
        nc.sync.dma_start(csta[:], cstA[:, :])
        nc.sync.dma_start(wva[:], wvA[:, :])
        nc.sync.dma_start(ones_t[:], onesT[:, :])
        nc.sync.dma_start(csa[:, 512:S], csA[:, 512:S])
        nc.sync.dma_start(csa[:, S + 512:2 * S], csA[:, S + 512:2 * S])
        nc.sync.dma_start(woa[:], woA[:, :])

        psum = ctx.enter_context(tc.tile_pool(name="psum", bufs=2,
                                              space="PSUM"))
        atp = ctx.enter_context(tc.tile_pool(name="atp", bufs=4))
        dvp = ctx.enter_context(tc.tile_pool(name="dvp", bufs=3))
        osb = ctx.enter_context(tc.tile_pool(name="osb", bufs=2))
        rp = ctx.enter_context(tc.tile_pool(name="rope", bufs=4))

        # PSUM budget (8 banks): "sc" [128,1024] fp32 x2 bufs = 4 banks
        # (score pairs, filler chains, regroup); "ot0"/"ot1" [128,512] x2
        # bufs = 4 banks (O^T accumulators, stage-A projection chains).
        def sc_tile(name):
            return psum.tile([128, 1024], F32, tag="sc", name=name, bufs=2)

        def ot_tile(h, name):
            return psum.tile([128, 512], F32, tag=f"ot{h % 2}", name=name,
                             bufs=2)

        # ---- building blocks -------------------------------------------
        def qk_group(dst, wmat, m, nch):
            ps = sc_tile("qkg")
            for k in range(KD):
                nc.tensor.matmul(
                    ps[:, 0:512],
                    wmat[:, 256 * k + 128 * m:256 * k + 128 * (m + 1)],
                    xt[k][:, 512 * nch:512 * (nch + 1)],
                    start=(k == 0), stop=(k == KD - 1))
            nc.scalar.copy(dst[m][:, 512 * nch:512 * (nch + 1)], ps[:, 0:512])

        def v_group(s, ps=None, sl=slice(0, HF)):
            nc.gpsimd.memset(v_sb[s][:], 1.0)
            if ps is None:
                ps = sc_tile("vg")
                sl = slice(0, HF)
            for k in range(KD):
                nc.tensor.matmul(
                    ps[:, sl], xt[k][:, 128 * s:128 * (s + 1)],
                    wva[:, 256 * k:256 * (k + 1)],
                    start=(k == 0), stop=(k == KD - 1))
            nc.scalar.copy(
                v_sb[s][:].rearrange(
                    "p (h c) -> p h c", c=DK + 1)[:, :, 0:DK],
                ps[:, sl].rearrange("p (h c) -> p h c", c=DK))

        def rope(nch):
            sl = bass.ts(nch, 512)
            for fe, fo, be, bo in ((qf[0], qf[1], qte, qto),
                                   (kf[0], kf[1], kte, kto)):
                t1 = rp.tile([128, 512], BF16, tag="rt", name="t1")
                nc.vector.tensor_mul(t1[:], fe[:, sl], cos_t[:, sl])
                t2 = rp.tile([128, 512], BF16, tag="rt", name="t2")
                nc.vector.tensor_mul(t2[:], fo[:, sl], sin_t[:, sl])
                nc.vector.tensor_sub(be[:, sl], t1[:], t2[:])
                t3 = rp.tile([128, 512], BF16, tag="rt", name="t3")
                nc.vector.tensor_mul(t3[:], fe[:, sl], sin_t[:, sl])
                t4 = rp.tile([128, 512], BF16, tag="rt", name="t4")
                nc.vector.tensor_mul(t4[:], fo[:, sl], cos_t[:, sl])
                nc.vector.tensor_add(bo[:, sl], t3[:], t4[:])

        def regroup(nch):
            # even/odd halves -> head-contiguous [64f, 512] blocks, as a
            # pair of 0/1-permutation matmuls per destination tile (exact
            # in bf16, and much cheaper than per-head SBUF-SBUF DMAs)
            sl = bass.ts(nch, 512)
            jobs = [(dsts, i, se, so)
                    for dsts, se, so in ((qc, qte, qto), (kc, kte, kto))
                    for i in range(2)]
            for j, (dsts, i, se, so) in enumerate(jobs):
                if j % 2 == 0:
                    ps = sc_tile("rg")
                half = slice(512 * (j % 2), 512 * (j % 2) + 512)
                nc.tensor.matmul(ps[:, half], perm[2 * i][:], se[:, sl],
                                 start=True, stop=False)
                nc.tensor.matmul(ps[:, half], perm[2 * i + 1][:], so[:, sl],
                                 start=False, stop=True)
                nc.vector.tensor_copy(dsts[i][:, sl], ps[:, half])

        def divide(qr, ops_):
            # normalize O^T rows by the ones-column denominators
            for h, ot in ops_.items():
                o65 = dvp.tile([65, 512], F32, tag=f"o65{h % 2}",
                               name=f"o65{h % 2}")
                nc.vector.tensor_copy(o65[:], ot[0:65, :])
                rc = dvp.tile([1, 512], F32R, tag=f"rc{h % 2}",
                              name=f"rc{h % 2}")
                with nc.allow_low_precision("fp32r recip broadcast"):
                    nc.vector.reciprocal(rc[:], o65[64:65, :])
                # broadcast recip across 64 partitions on PE: K=1 matmul
                # against a ones column, into the already-copied O^T bank.
                # float32r streams at bf16 rate; the multiply is by 1.0.
                nc.tensor.matmul(ot[0:64, :], ones_t[:], rc[:],
                                 start=True, stop=True)
                nc.vector.tensor_mul(
                    otb[h // 2][64 * (h % 2):64 * (h % 2) + 64,
                                512 * qr:512 * (qr + 1)],
                    o65[0:64, :], ot[0:64, :])

        def outproj(qr):
            for sp_ in range(2):  # two 2-s-block slabs -> one DMA each
                ob = osb.tile([128, 2048], BF16, tag="ob", name="ob")
                jobs = [(sh, nch) for sh in range(2) for nch in range(2)]
                for j, (sh, nch) in enumerate(jobs):
                    s = 4 * qr + 2 * sp_ + sh
                    if j % 2 == 0:
                        ps = sc_tile("op")
                    half = slice(512 * (j % 2), 512 * (j % 2) + 512)
                    for i2 in range(2):
                        nc.tensor.matmul(
                            ps[:, half], otb[i2][:, 128 * s:128 * (s + 1)],
                            woa[:, 1024 * i2 + 512 * nch:
                                 1024 * i2 + 512 * (nch + 1)],
                            start=(i2 == 0), stop=(i2 == 1))
                    nc.scalar.copy(
                        ob[:, 1024 * sh + 512 * nch:
                           1024 * sh + 512 * (nch + 1)], ps[:, half])
                s0 = 4 * qr + 2 * sp_
                nc.sync.dma_start(
                    out[128 * s0:128 * (s0 + 2), :].rearrange(
                        "(a p) d -> p a d", p=128),
                    ob[:])

        # ---- stage A ---------------------------------------------------
        # Full Q projection runs progressively as x chunks stream in (PE is
        # otherwise DMA-starved at the start); all 8 PSUM banks hold the 8
        # (m, nch) accumulators until x is resident.
        qaccs = []
        for t in range(2):
            tq = sc_tile(f"qa{t}")
            qaccs += [(tq, slice(0, 512)), (tq, slice(512, 1024))]
        for t in range(4):
            qaccs.append((ot_tile(t % 2, f"qov{t}"), slice(0, 512)))
        for k in range(KD):
            for m in range(2):
                for nch in range(NS):
                    tq, sl_ = qaccs[4 * m + nch]
                    nc.tensor.matmul(
                        tq[:, sl_],
                        wqa[:, 256 * k + 128 * m:256 * k + 128 * (m + 1)],
                        xt[k][:, 512 * nch:512 * (nch + 1)],
                        start=(k == 0), stop=(k == KD - 1))
        for m in range(2):
            for nch in range(NS):
                tq, sl_ = qaccs[4 * m + nch]
                nc.scalar.copy(qf[m][:, 512 * nch:512 * (nch + 1)],
                               tq[:, sl_])
        # K projection + V for the first q-range, then RoPE/regroup(0)
        for m in range(2):
            qk_group(kf, wka, m, 0)
        rope(0)
        regroup(0)
        for s in range(4):
            v_group(s, ps=ot_tile(s % 2, f"vg{s}"), sl=slice(0, HF))

        # remaining projection work (K nch>=1, V s>=4, rope, regroup),
        # flushed at q-range boundaries where the score-tile double-buffer
        # is momentarily idle.
        fillers = []
        for nch in range(1, NS):
            for m in range(2):
                fillers.append((nch, lambda m=m, n=nch:
                                qk_group(kf, wka, m, n)))
            for s in range(4 * nch, 4 * (nch + 1)):
                fillers.append((nch, lambda s=s: v_group(s)))
            fillers.append((nch, lambda n=nch: rope(n)))
            fillers.append((nch, lambda n=nch: regroup(n)))

        def pop_fillers(qr):
            # flush everything qr is about to read
            while fillers and fillers[0][0] <= qr:
                fillers.pop(0)[1]()

        def pop_one_filler(before_qr):
            # opportunistically run one filler item mid-attention (between a
            # kb's score MMs and its AV MMs) so PE never starves while the
            # Exp for that kb is in flight
            if fillers and fillers[0][0] <= before_qr:
                fillers.pop(0)[1]()

        # ---- attention + out-projection, software-pipelined ------------
        pending = [None]
        for qr in range(NS):
            q0 = 512 * qr
            # everything this q-range reads must be emitted already
            pop_fillers(qr)
            for hp in range(2):
                heads = (2 * hp, 2 * hp + 1)
                ops_ = {h: ot_tile(h, f"ot{h % 2}") for h in heads}
                nkb = 4 * (qr + 1)

                def scores_exp(kb):
                    # both heads' transposed scores in one 2-bank PSUM tile,
                    # then ONE Exp for the pair (overhead halves)
                    off = max(0, 128 * kb - q0)
                    sc2 = sc_tile("sc")
                    for h in heads:
                        po = 64 * (h % 2)
                        nc.tensor.matmul(
                            sc2[:, 512 * (h % 2) + off:512 * (h % 2) + 512],
                            kc[h // 2][po:po + 64,
                                       128 * kb:128 * (kb + 1)],
                            qc[h // 2][po:po + 64, q0 + off:q0 + 512],
                            start=True, stop=True,
                            tile_position=(po, 0),
                            skip_group_check=True)
                    at2 = atp.tile([128, 1024], BF16, tag="at", name="at2")
                    nc.scalar.activation(
                        at2[:].rearrange("p (t c) -> p t c", t=2)[:, :,
                                                                  off:512],
                        sc2[:].rearrange("p (t c) -> p t c", t=2)[:, :,
                                                                  off:512],
                        AF.Exp, scale=SCALE)
                    if kb >= 4 * qr:  # diagonal block: causal mask
                        for h in heads:
                            hh = 512 * (h % 2)
                            nc.gpsimd.tensor_mul(
                                at2[:, hh + off:hh + off + 128],
                                at2[:, hh + off:hh + off + 128],
                                tri_t[:])
                    return at2, off

                def av(kb, at2, off):
                    for h in heads:
                        hh = 512 * (h % 2)
                        nc.tensor.matmul(
                            ops_[h][0:65, off:512],
                            v_sb[kb][:, 65 * h:65 * h + 65],
                            at2[:, hh + off:hh + 512],
                            start=(kb == 0), stop=(kb == nkb - 1))

                # software-pipelined: scores/exp run one kb ahead of the AV
                # consumers so the exp latency never heads the PE queue
                prev = scores_exp(0)
                for kb in range(1, nkb):
                    nxt = scores_exp(kb)
                    av(kb - 1, *prev)
                    prev = nxt
                av(nkb - 1, *prev)
                # emit the PREVIOUS pair's division (one-pair delay so its
                # DVE chain never stalls the PE stream)
                if pending[0] is not None:
                    pqr, php, pops = pending[0]
                    divide(pqr, pops)
                    if php == 1:
                        outproj(pqr)
                pending[0] = (qr, hp, ops_)

        pqr, php, pops = pending[0]
        divide(pqr, pops)
        outproj(pqr)

    if not nc.is_finalized():
        nc.finalize()
    return nc


_CACHE = {}


def _get_nc():
    if "nc" not in _CACHE:
        _CACHE["nc"] = _build_program()
    return _CACHE["nc"]


# ---------------------------------------------------------------------------
# Host side: shard, run, gather
# ---------------------------------------------------------------------------
def _pack_w(w):
    # [1024, 256] -> SBUF-wide [128, 8*256] (k-chunks side by side)
    return np.ascontiguousarray(
        w.reshape(KD, 128, HF).transpose(1, 0, 2).reshape(128, KD * HF))


def _core_inputs(c, x, Wq, Wk, Wv, Wo, csA, cstA):
    b, hg = c // 4, c % 4
    bf = ml_dtypes.bfloat16
    xTc = np.ascontiguousarray(x[b].T).astype(bf)
    # feature permutation: [evens of h0..h3 | odds of h0..h3]
    rows = []
    for par in (0, 1):
        for j in range(HPC):
            base = DK * (HPC * hg + j)
            rows += [base + 2 * i + par for i in range(DK // 2)]
    rows = np.asarray(rows)
    vcols = np.arange(HF) + HF * hg
    woTc = np.ascontiguousarray(Wo[:, vcols].T)  # [256, 1024]
    return {
        "xT": xTc,
        "wqA": _pack_w(Wq[rows, :].T.astype(bf)),
        "wkA": _pack_w(Wk[rows, :].T.astype(bf)),
        "wvA": _pack_w(Wv[vcols, :].T.astype(bf)),
        "woA": np.ascontiguousarray(
            woTc.reshape(2, 128, D).transpose(1, 0, 2).reshape(128, 2 * D)
        ).astype(bf),
        "csA": csA, "cstA": cstA,
        "onesT": np.ones((1, 64), np.float32),
    }


def _run(x, Wq, Wk, Wv, Wo, token_positions, **spmd_kwargs):
    x = np.asarray(x, np.float32)
    Wq = np.asarray(Wq, np.float32)
    Wk = np.asarray(Wk, np.float32)
    Wv = np.asarray(Wv, np.float32)
    Wo = np.asarray(Wo, np.float32)
    pos = np.asarray(token_positions).astype(np.float32)

    inv = THETA ** (-np.arange(0, DK, 2, dtype=np.float32) / DK)  # [32]
    ang = pos[:, None] * inv[None, :]                             # [S, 32]
    cosT = np.tile(np.cos(ang).T, (4, 1))                         # [128, S]
    sinT = np.tile(np.sin(ang).T, (4, 1))
    csA = np.ascontiguousarray(
        np.concatenate([cosT, sinT], axis=1)).astype(ml_dtypes.bfloat16)

    kk, qq = np.meshgrid(np.arange(128), np.arange(128), indexing="ij")
    tri = (kk <= qq).astype(np.float32)                           # [k, q]
    # 0/1 permutation mats: [even/odd, dst-tile] -> head-contiguous rows
    perms = np.zeros((2, 2, 128, 128), np.float32)
    for par in range(2):
        for i in range(2):
            for r in range(64):
                src = 64 * i + r
                dst = 64 * (r // 32) + 32 * par + (r % 32)
                perms[par, i, src, dst] = 1.0
    cstA = np.ascontiguousarray(np.concatenate(
        [tri, perms[0, 0], perms[1, 0], perms[0, 1], perms[1, 1]],
        axis=1)).astype(ml_dtypes.bfloat16)

    in_maps = [_core_inputs(c, x, Wq, Wk, Wv, Wo, csA, cstA)
               for c in range(N_CORES)]
    res = run_bass_kernel_spmd(_get_nc(), in_maps,
                               core_ids=list(range(N_CORES)), **spmd_kwargs)
    outf = np.zeros((B, S, D), np.float32)
    for c in range(N_CORES):
        outf[c // 4] += np.asarray(res.results[c]["out"], np.float32)
    return outf, res


def kernel(x, Wq, Wk, Wv, Wo, token_positions):
    outf, _ = _run(x, Wq, Wk, Wv, Wo, token_positions)
    return outf

